# revision 15
# baseline (speedup 1.0000x reference)
"""Trainium2 Bass kernel for the additive-attention module.

Reference math (single device):
    enc    = einsum('sbh,kh->sbk', encoder_output, We) + be     # [S,B,K]
    hid    = hidden @ Wh.T + bh                                 # [B,K]
    energy = sigmoid(enc + hid[None]) @ Wv.T + bv               # [S,B,1]
    attn   = softmax(energy, axis=0)                            # over S
    out    = attn.transpose(1, 2, 0)                            # [B,1,S]

Device strategy (8 NeuronCores, data-parallel over batch):
  * Each core gets 8 of the 64 batches; weights replicated.
  * encoder_output is re-laid-out on the host to [H, B_core, S] so the
    contraction dim H lands on SBUF partitions (b-major so the per-batch
    hid term is a per-partition ACT bias).  The fp32 HBM data is cast to
    bf16 in-flight by the SWDGE DMA.
  * sigmoid(x) = (1 + tanh(x/2))/2, and softmax is invariant to the
    affine constants, so on device we compute
        E[s,b] = sum_k Wv[k] * tanh(0.5*enc_raw + hidb)
    (hidb = 0.5*(hidden @ Wh.T + bh + be), computed on host - 17 MFLOP)
    and finish with softmax(0.5 * E).  tanh shares the ACT table set
    with exp, so there is no table thrashing.
  * The Wv reduction over k rides the PE as matmuls with a zero-padded
    [128,128] stationary operand (column 0 = Wv chunk) so the weight
    load takes the FWL path and hides under the previous matmul.
"""

import os
import numpy as np

import concourse.bass as bass
import concourse.mybir as mybir
import concourse.tile as tile
from concourse import bacc
from concourse.bass_utils import run_bass_kernel_spmd

S_TOT = 4096
B_TOT = 64
H = 512
N_CORES = 8
BPC = B_TOT // N_CORES  # batches per core
P = 128
KC = H // P  # 4 contraction / output chunks
SH = 1024    # s-chunk processed per tanh tile
NMM = 512    # matmul moving free dim

F32 = mybir.dt.float32
BF16 = mybir.dt.bfloat16
F8 = mybir.dt.float8e4
WE_SCALE = 32.0

# Results of the most recent device run (for the local test harness only).
LAST_RESULTS = None

_BUILD_CACHE = {}


def _build(s_tot=S_TOT, bpc=BPC, n_cores=N_CORES):
    key = (s_tot, bpc, n_cores)
    if key in _BUILD_CACHE:
        return _BUILD_CACHE[key]

    nc = bacc.Bacc(
        "TRN2", target_bir_lowering=False, debug=False, num_devices=n_cores
    )
    eoT = nc.dram_tensor("eoT", [H, bpc, s_tot], F32, kind="ExternalInput")
    WeT = nc.dram_tensor("WeT", [P, KC, H], F8, kind="ExternalInput")
    hidb = nc.dram_tensor("hidb", [P, KC, bpc], F32, kind="ExternalInput")
    Wvp = nc.dram_tensor("Wvp", [P, KC * P], BF16, kind="ExternalInput")
    out = nc.dram_tensor("out", [bpc, s_tot], F32, kind="ExternalOutput")

    sh = min(SH, s_tot)
    nns = sh // NMM
    Tanh = mybir.ActivationFunctionType.Tanh
    Exp = mybir.ActivationFunctionType.Exp

    with tile.TileContext(nc) as tc:
        with (
            tc.tile_pool(name="weights", bufs=1) as wpool,
            tc.tile_pool(name="ebuf", bufs=6) as epool,
            tc.tile_pool(name="sig", bufs=4) as sigpool,
            tc.tile_pool(name="estage", bufs=2) as stpool,
            tc.tile_pool(name="small", bufs=1) as spool,
            tc.tile_pool(name="enc", bufs=2, space="PSUM") as encpool,
            tc.tile_pool(name="epsum", bufs=4, space="PSUM") as enpool,
        ):
            WeT_sb = wpool.tile([P, KC, H], F8, tag="WeT")
            nc.sync.dma_start(WeT_sb[:], WeT.ap())
            hidb_sb = wpool.tile([P, KC, bpc], F32, tag="hidb")
            nc.sync.dma_start(hidb_sb[:], hidb.ap())
            Wv_sb = wpool.tile([P, KC * P], BF16, tag="Wvp")
            nc.sync.dma_start(Wv_sb[:], Wvp.ap())

            energy_sb = spool.tile([bpc, s_tot], F32, tag="energy")
            pexp = spool.tile([bpc, s_tot], F32, tag="pexp")
            eoT_r = eoT.ap().rearrange("(c p) b s -> p c b s", p=P)

            s_blk = min(SH, s_tot)
            nblk = s_tot // s_blk
            # per-block softmax partials (online softmax over s)
            sloc = spool.tile([bpc, nblk], F32, tag="sloc")   # rowsum(exp) per blk
            for blk in range(nblk):
                sb0 = blk * s_blk
                for b in range(bpc):
                    ebuf = epool.tile([P, KC, s_blk], F8, tag="ebuf")
                    nc.gpsimd.dma_start(
                        ebuf[:], eoT_r[:, :, b, sb0:sb0 + s_blk]
                    )
                    stage = stpool.tile([1, s_blk], F32, tag="estage")
                    eps = [
                        enpool.tile([P, NMM], F32, tag="epsum", name=f"eps{ns}")
                        for ns in range(nns)
                    ]
                    for kc in range(KC):
                        enc = encpool.tile([P, sh], F32, tag="enc")
                        for ns in range(nns):
                            s0 = ns * NMM
                            for hc in range(0, KC, 2):
                                nc.tensor.matmul(
                                    enc[:, s0:s0 + NMM],
                                    WeT_sb[:, hc:hc + 2, kc * P:(kc + 1) * P],
                                    ebuf[:, hc:hc + 2, s0:s0 + NMM],
                                    start=(hc == 0),
                                    stop=(hc == KC - 2),
                                    perf_mode=mybir.MatmulPerfMode.DoubleRow,
                                )
                        sig = sigpool.tile([P, sh], BF16, tag="sig")
                        nc.scalar.activation(
                            sig[:], enc[:], Tanh,
                            scale=0.5 / WE_SCALE, bias=hidb_sb[:, kc, b:b + 1],
                        )
                        for ns in range(nns):
                            nc.tensor.matmul(
                                eps[ns][:, :],
                                Wv_sb[:, kc * P:(kc + 1) * P],
                                sig[:, ns * NMM:(ns + 1) * NMM],
                                start=(kc == 0),
                                stop=(kc == KC - 1),
                            )
                    for ns in range(nns):
                        nc.vector.tensor_copy(
                            stage[0:1, ns * NMM:(ns + 1) * NMM], eps[ns][0:1, :]
                        )
                    nc.sync.dma_start(
                        energy_sb[b:b + 1, sb0:sb0 + s_blk], stage[:]
                    )
                # partial softmax stats for this block (runs under next
                # block).  |0.5*E| <= 0.5*sum|Wv| ~ 5.7, so exp cannot
                # overflow fp32 and no max-subtraction is needed.
                eblk = energy_sb[:, sb0:sb0 + s_blk]
                nc.scalar.activation(
                    pexp[:, sb0:sb0 + s_blk], eblk, Exp, scale=0.5,
                )
                nc.vector.tensor_reduce(
                    out=sloc[:, blk:blk + 1], in_=pexp[:, sb0:sb0 + s_blk],
                    axis=mybir.AxisListType.X, op=mybir.AluOpType.add,
                )

            # attn = pexp / rowsum(pexp)
            stot = spool.tile([bpc, 1], F32, tag="stot")
            nc.vector.tensor_reduce(
                out=stot[:], in_=sloc[:],
                axis=mybir.AxisListType.X, op=mybir.AluOpType.add,
            )
            rec = spool.tile([bpc, 1], F32, tag="rec")
            nc.vector.reciprocal(rec[:], stot[:])
            for blk in range(nblk):
                sb0 = blk * s_blk
                nc.vector.tensor_scalar_mul(
                    pexp[:, sb0:sb0 + s_blk], pexp[:, sb0:sb0 + s_blk],
                    rec[:],
                )
            nc.sync.dma_start(out.ap()[:, :], pexp[:])

    nc.compile()
    _BUILD_CACHE[key] = nc
    return nc


def make_in_maps(hidden, encoder_output, We, be, Wh, bh, Wv):
    """Host-side sharding/layout prep. Returns per-core input dicts."""
    import ml_dtypes
    eo = np.ascontiguousarray(np.asarray(encoder_output, dtype=np.float32))
    hidden = np.asarray(hidden, dtype=np.float32)
    WeT = np.ascontiguousarray(
        (np.asarray(We, np.float32).T * WE_SCALE)
        .reshape(KC, P, H).transpose(1, 0, 2)
    ).astype(ml_dtypes.float8_e4m3fn)  # [P, KC(hc), H(k)]

    # hidb = 0.5 * (hidden @ Wh.T + bh + be), laid out [P, KC, B]
    hid_all = 0.5 * (
        hidden @ np.asarray(Wh, np.float32).T
        + np.asarray(bh, np.float32) + np.asarray(be, np.float32)
    )  # [B_TOT, H]
    # Wv padded stationary operand: [P, KC*P], column 0 of each kc block
    Wvp = np.zeros((P, KC * P), np.float32)
    wv = np.asarray(Wv, np.float32).reshape(-1)  # [H]
    for kc in range(KC):
        Wvp[:, kc * P] = wv[kc * P:(kc + 1) * P]
    Wvp = Wvp.astype(ml_dtypes.bfloat16)

    in_maps = []
    for c in range(N_CORES):
        b0 = c * BPC
        eoT_c = np.ascontiguousarray(
            eo[:, b0:b0 + BPC, :].transpose(2, 1, 0)
        )  # [H, BPC, S]
        hidb_c = np.ascontiguousarray(
            hid_all[b0:b0 + BPC].T.reshape(KC, P, BPC).transpose(1, 0, 2)
        )  # [P, KC, BPC]
        in_maps.append({
            "eoT": eoT_c,
            "WeT": WeT,
            "hidb": hidb_c,
            "Wvp": Wvp,
        })
    return in_maps


def kernel(hidden, encoder_output, each_size=None, We=None, be=None,
           Wh=None, bh=None, Wv=None, bv=None):
    global LAST_RESULTS
    nc = _build()
    in_maps = make_in_maps(hidden, encoder_output, We, be, Wh, bh, Wv)
    res = run_bass_kernel_spmd(
        nc, in_maps, list(range(N_CORES)),
        trace=bool(os.environ.get("BASS_TRACE")),
    )
    LAST_RESULTS = res
    attn = np.concatenate([res.results[c]["out"] for c in range(N_CORES)], axis=0)
    return np.ascontiguousarray(attn.reshape(B_TOT, 1, S_TOT).astype(np.float32))
